# revision 43
# baseline (speedup 1.0000x reference)
"""Trainium2 Bass kernel for nn_MultiHeadAttention (B=2, S=2048, D=1024, H=16).

Sharding: 8 cores = 2 (batch) x 4 (head groups of 4 heads / 256 dims).
Each core computes QKV projections for its head slice, attention for its 4
heads, and the partial output projection for its 256-dim slice of Wo's input.
Host sums the 4 bf16 partials per batch element (Megatron-style row-parallel
Wo).

All device inputs are pre-packed host-side into exact SBUF tile layout so
every DMA is a straight contiguous copy (8-16KB lines, minimal descriptor
count).  DMA issue order + a data-dependency gate on the second wave
(wv/v/mask/wo behind kproj(0)) keep the round-robin DMA engines focused on
the k/q blocks the head needs first.

Device layouts (per core):
  qT/kT/vT  packed [sb*128+p, c*512+s] bf16
  maskT     packed [sb*128+p, c*512+s] bf16 (0.0/1.0), DMA'd per t-half
  qpT/kpT   [256(j), 2048(s)]   (projections, transposed: j on partitions)
  vp        [2048(t), 4x65]     (natural layout; col 64 of each 65-block = 1.0
                                 -> attn@V matmul also produces softmax denom)
  P~        [t, s] = exp(scoresT/8) * maskT   (scoresT = K_h.T^T @ Q_h.T)
  attn out  [65(j+denom), s] -> normalized -> concatT [256(j), 2048(s)]
  out_p     [2048, 1024] bf16 partial = concatT.T @ woT

Pipeline per (sb, pair) iteration (16 t-chunks): scores (2 concurrent K=64
row-group matmuls) -> exp on ACT (the pacing engine) -> quarter-granular
mask-mul on DVE -> prev iteration's attnV interleaved 1-per-slot (keeps the
PE dense enough to hold HAM at 8/8; per-chunk masks in the last iteration's
final quarter let its attnV finish in-loop).  Norm = fused denominator from
the V ones-column, batched reciprocal + single partition_broadcast;
projection bias-adds and the tail Wo copies ride the ramp/tail-idle ACT.
"""

import sys

import numpy as np

try:
    import concourse.bass as bass
except ImportError:  # pragma: no cover
    sys.path.insert(0, "/opt/trn_rl_repo")
    import concourse.bass as bass

from concourse import bacc

import ml_dtypes

import concourse.tile as tile_mod
from concourse import mybir
from concourse.bass_utils import run_bass_kernel_spmd

BF16 = ml_dtypes.bfloat16
F32 = np.float32

B, S, D, H = 2, 2048, 1024, 16
DK = D // H            # 64
N_CORES = 8
HPC = 4                # heads per core
JC = HPC * DK          # 256 j-dims per core
SCALE = 1.0 / float(np.sqrt(DK))
NSB = S // 512         # 4 s-blocks
NC_T = S // 128        # 16 t-chunks
VROW = HPC * 65        # 260: [h0 64 | 1 | h1 64 | 1 | ...]

bf = mybir.dt.bfloat16
f32 = mybir.dt.float32
f8 = mybir.dt.bfloat16   # fp8 q/k inputs tried and reverted: logit noise 4.5e-2
F8 = BF16
e4 = mybir.dt.float8e4   # fp8 e4m3: OK on scores path only; v-path fp8 costs 4e-2 rel err
E4 = ml_dtypes.float8_e4m3
WPRE = 1.0
WINV = 1.0 / WPRE


def _patch_drain():
    """This walrus build only accepts 1 sync-wait per instruction; the Tile
    exit drain carries one wait per pending proc. Split them across drains."""
    if getattr(tile_mod.TileContext, "_drain_patched", False):
        return
    import bass_rust

    def _drain_and_barrier(self, tick_clock, wait_clock):
        from concourse.tile import ScopedClock

        nc = self.nc
        drain_inst = nc.sync.drain()
        wait_clock.add_sem_waits(
            drain_inst.ins, ScopedClock({None: tick_clock.global_clock})
        )
        si = drain_inst.ins.sync_info
        waits = list(si.on_wait)
        if len(waits) > 1:
            drain_inst.ins.sync_info = bass_rust.SyncInfo(
                on_wait=[waits[0]], on_update=list(si.on_update)
            )
            for w in waits[1:]:
                d2 = nc.sync.drain()
                d2.ins.sync_info = bass_rust.SyncInfo(on_wait=[w], on_update=[])
        nc.all_engine_barrier()
        assert self.sems is not None
        popped = nc._tile_sem_poison_stack.pop()
        assert popped is self._sem_poison
        nc.clear_and_free_semaphores(list(self.sems.allocated().values()))
        nc.all_engine_barrier()

    tile_mod.TileContext._drain_and_barrier = _drain_and_barrier
    tile_mod.TileContext._drain_patched = True


ALPHA = SCALE * float(np.log2(np.e)) * 128.0   # exp2 bit-hack scale, folded
                                               # into the k-projection output
EXP_SCALE = 1.0 / (float(np.log2(np.e)) * 128.0)  # ACT exp scale on the
                                                  # ALPHA-prescaled scores
# ACT-route log-bias matching the bithack's mean pwl overestimate
# E[ln((1+f)/2^f)] = (2ln2-1) - ln2/2: makes the route bias cancel in softmax
ACT_COMP = float(2 * np.log(2) - 1 - np.log(2) / 2)
# Per-iteration chunk routing for the P = exp(scores)*mask production.
# DVE_C: fused on DVE as bits=int16(psum + maskB) reinterpreted bf16
#        (exp2 piecewise-linear bit-hack, mask folded additively).
# POOL_C: ACT exp, then mask-mul on the (otherwise idle) Pool engine.
# Rest:  ACT exp + DVE mask-mul, grouped per contiguous plain run.
DVE_C = (4, 8, 12)
POOL_C = ()


def _mk_groups(dve_c, pool_c):
    plain = [c for c in range(NC_T) if c not in dve_c and c not in pool_c]
    groups, start = {}, None
    for i, c in enumerate(plain):
        if start is None:
            start = c
        if i + 1 >= len(plain) or plain[i + 1] != c + 1:
            groups[c] = (start, c + 1)
            start = None
    return plain, groups


def routes_for(sb):
    return (DVE_C, POOL_C)


_plain_0, PLAIN_GROUPS_0 = _mk_groups((), ())
_plain_n, PLAIN_GROUPS_N = _mk_groups(DVE_C, POOL_C)


def _emit(tc, T):
    nc = tc.nc
    Exp = mybir.ActivationFunctionType.Exp
    i16 = mybir.dt.int16

    from contextlib import ExitStack

    with ExitStack() as ctx:
        persist = ctx.enter_context(tc.tile_pool(name="persist", bufs=1))

        # ---- weights / persistent tiles ----
        # DMA issue order matters: the DMA engines round-robin across active
        # queues, so anything issued early competes with the k/q blocks the
        # head needs first.  Sync queue: wk, wq, biasqk, kT0, qT0, kT1...
        # The wv/wo/mask/v issues are gated behind kproj(0) completion via a
        # dummy gpsimd read of kpT (emitted later).
        wq = persist.tile([128, 8 * JC], f8, tag="wq")
        wk = persist.tile([128, 8 * JC], f8, tag="wk")
        wv = persist.tile([128, 8 * JC], f8, tag="wv")
        for t, name in ((wk, "wkT"), (wq, "wqT")):
            nc.sync.dma_start(t[:], T[name][:, :])
        biasqk = persist.tile([128, 5], f32, tag="biasqk")
        nc.sync.dma_start(biasqk[:], T["biasqk"][:, :])
        identf = persist.tile([128, 128], f32, tag="identf")
        nc.sync.dma_start(identf[:], T["ident"][:, :])
        wo = [persist.tile([128, D], bf, tag=f"wo{i}", name=f"wo{i}") for i in range(2)]

        # per-sb q/k projection tiles ([j, s] transposed layout)
        qpS = [
            [persist.tile([128, 512], bf, tag=f"qp{j}_{s}", name=f"qp{j}_{s}")
             for s in range(NSB)]
            for j in range(2)
        ]
        kpT = [
            [persist.tile([128, 1024], bf, tag=f"kpT{i}_{th}", name=f"kpT{i}_{th}")
             for th in range(2)]
            for i in range(2)
        ]
        # per-chunk v tiles (natural [t, j] layout + ones cols)
        vpc = [persist.tile([128, VROW], bf, tag=f"vp{c}", name=f"vp{c}")
               for c in range(NC_T)]
        concatT = [persist.tile([128, S], bf, tag=f"concatT{i}", name=f"concatT{i}") for i in range(2)]

        wq_v = wq[:].rearrange("p (c j) -> p c j", c=8)
        wk_v = wk[:].rearrange("p (c j) -> p c j", c=8)
        wv_v = wv[:].rearrange("p (c j) -> p c j", c=8)

        q_stream = ctx.enter_context(tc.tile_pool(name="q_stream", bufs=1))
        qtts = {}

        def emit_qdma(sb):
            sl = slice(sb * 512, (sb + 1) * 512)
            qTt = q_stream.tile([128, 8 * 512], f8, tag="qTt", name=f"qTt{sb}")
            nc.sync.dma_start(qTt[:], T["qT"][sb * 128 : (sb + 1) * 128, :])
            qtts[sb] = qTt[:].rearrange("p (c s) -> p c s", c=8)

        def emit_qproj_jt(sb, jt):
            jsl = slice(jt * 128, (jt + 1) * 128)
            ps = bigp.tile([128, 512], f32, tag="big", name=f"pq{sb}_{jt}")
            for c in range(8):
                nc.tensor.matmul(
                    ps[:], wq_v[:, c, jsl], qtts[sb][:, c, :],
                    start=(c == 0), stop=(c == 7),
                )
            # on ACT: keeps the DVE queue short so it never gates scores-psum
            # recycling (the bit-hack chunks consume psum on DVE)
            nc.scalar.activation(
                qpS[jt][sb][:], ps[:], mybir.ActivationFunctionType.Identity,
                bias=biasqk[:, jt : jt + 1], scale=WINV,
            )

        def emit_qproj(sb):
            emit_qdma(sb)
            emit_qproj_jt(sb, 0)
            emit_qproj_jt(sb, 1)

        # ---- attention + output projection ----
        # Chunk-level software pipeline: per t-chunk the PE stream carries
        # scores(i) for both heads (concurrent row-groups), then attnV(i-1)
        # for both heads, plus occasional "extras" (Wo / q-proj / v-proj /
        # mask prefetch). ACT (exp) is the pacing engine; this keeps it fed
        # every chunk while the PE stays dense enough to hold HAM at 8/8.
        if True:
            vstream = ctx.enter_context(tc.tile_pool(name="vstream", bufs=4))
            schp = ctx.enter_context(tc.tile_pool(name="schp", bufs=2))
            maskp = ctx.enter_context(tc.tile_pool(name="maskp", bufs=2))
            ptp = ctx.enter_context(tc.tile_pool(name="ptp", bufs=2))
            smallp = ctx.enter_context(tc.tile_pool(name="smallp", bufs=1))
            cnp = ctx.enter_context(tc.tile_pool(name="cnp", bufs=2))
            outp = ctx.enter_context(tc.tile_pool(name="outp", bufs=4))
            scp = ctx.enter_context(tc.tile_pool(name="scp", bufs=2, space="PSUM"))
            mtiles = {}

        # ---- k/q projections pipelined into the attention loop ----
        # PSUM budget: scp 2x4KB + bigp 3x2KB + tpp 2x256B <= 16KB/partition
        bigp = ctx.enter_context(tc.tile_pool(name="bigp", bufs=3, space="PSUM"))
        tpp = ctx.enter_context(tc.tile_pool(name="tpp", bufs=1, space="PSUM"))
        with tc.tile_pool(name="kv_stream", bufs=2) as kv_stream:
            ktts = {}

            def emit_kdma(sb):
                sl = slice(sb * 512, (sb + 1) * 512)
                kTt = kv_stream.tile([128, 8 * 512], f8, tag="kTt", name=f"kTt{sb}")
                nc.sync.dma_start(kTt[:], T["kT"][sb * 128 : (sb + 1) * 128, :])
                ktts[sb] = kTt[:].rearrange("p (c s) -> p c s", c=8)

            def emit_kproj_jt(sb, jt):
                jsl = slice(jt * 128, (jt + 1) * 128)
                ps = bigp.tile([128, 512], f32, tag="big", name=f"pk{sb}_{jt}")
                for c in range(8):
                    nc.tensor.matmul(
                        ps[:], wk_v[:, c, jsl], ktts[sb][:, c, :],
                        start=(c == 0), stop=(c == 7),
                    )
                # ACT is idle during the ramp where k-proj runs.  ALPHA is
                # folded in here (host pre-scales the k bias columns) so the
                # scores psum arrives pre-scaled for the exp2 bit-hack.
                nc.scalar.activation(
                    kpT[jt][sb // 2][:, (sb % 2) * 512 : (sb % 2 + 1) * 512],
                    ps[:], mybir.ActivationFunctionType.Identity,
                    bias=biasqk[:, 2 + jt : 3 + jt], scale=WINV * ALPHA,
                )

            # Head: get k0/q0 in flight first, project them, then start
            # attention immediately; k1-k3 projections run as extras inside
            # the (0,0) chunk loop (the PE is otherwise idle there since
            # there is no previous attnV to overlap).
            emit_kdma(0)
            emit_qdma(0)
            emit_kdma(1)
            emit_kproj_jt(0, 0)
            emit_kproj_jt(0, 1)
            emit_qproj_jt(0, 0)
            emit_qproj_jt(0, 1)
            emit_kdma(2)
            emit_kdma(3)
            # Gate the second DMA wave (wv/v/mask/wo) behind kproj(0) so the
            # round-robin DMA engines give the head's k/q blocks full
            # bandwidth first.  Tile schedules by data dependency (not
            # program order), so each gated DMA needs a REAL dep: write a
            # corner of its destination tile from kpT first (WAW ordering).
            def gate(dst_corner):
                nc.gpsimd.tensor_copy(dst_corner, kpT[0][0][0:1, 0:2])

            gate(wv[0:1, 0:2])
            nc.gpsimd.dma_start(wv[:], T["wvT"][:, :])




            def emit_mask_dma(sb, half, gated=False):
                # split per t-half so the piece needed by the c==7 mask-mul
                # lands first
                sl = slice(sb * 512, (sb + 1) * 512)
                if half == 0:
                    mT = maskp.tile(
                        [128, NC_T * 512], bf, tag="mT", name=f"mT{sb}"
                    )
                    mtiles[sb] = mT
                mT = mtiles[sb]
                if gated:
                    gate(mT[0:1, half * 4096 : half * 4096 + 2])
                # gated (ramp) issues ride Pool; mid-loop prefetches ride the
                # idle Sync queue so they never sit behind Pool mask-muls
                eng = nc.gpsimd if gated else nc.sync
                eng.dma_start(
                    mT[:, half * 4096 : (half + 1) * 4096],
                    T["maskT"][
                        sb * 128 : (sb + 1) * 128,
                        half * 4096 : (half + 1) * 4096,
                    ],
                )

            vtts = {}

            def emit_vdma(tb, gated=False):
                for hf in range(2):
                    blk = tb * 2 + hf
                    vTt = vstream.tile(
                        [128, 8 * 256], f8, tag="vTt", name=f"vTt{blk}"
                    )
                    if gated:
                        gate(vTt[0:1, 0:2])
                    eng = nc.gpsimd if gated else nc.sync
                    eng.dma_start(
                        vTt[:], T["vT"][blk * 128 : (blk + 1) * 128, :]
                    )
                    vtts[blk] = vTt[:].rearrange("p (c t) -> p c t", c=8)

            def emit_vproj_tb(tb):
                if tb + 2 < NSB and (tb + 2) * 2 not in vtts:
                    emit_vdma(tb + 2)
                for tt in range(4):
                    chunk = tb * 4 + tt
                    ps = bigp.tile([128, 512], f32, tag="big", name=f"pv{chunk}")
                    vTt_v = vtts[tb * 2 + tt // 2]
                    t0 = (tt % 2) * 128
                    for c in range(8):
                        nc.tensor.matmul(
                            ps[:, 0:JC],
                            vTt_v[:, c, t0 : t0 + 128],
                            wv_v[:, c, :],
                            start=(c == 0), stop=(c == 7),
                        )
                    vt = vpc[chunk]
                    nc.gpsimd.memset(
                        vt[:].rearrange("p (h d) -> p h d", d=65)[:, :, 64:65],
                        1.0,
                    )
                    dst = vt[:].rearrange("p (h d) -> p h d", h=HPC)[:, :, 0:DK]
                    src = ps[:, 0:JC].rearrange("p (h d) -> p h d", h=HPC)
                    # ACT is idle during the ramp where vproj runs
                    nc.scalar.mul(dst, src, WINV)

            def emit_warm(pt, n=1):
                # dependency-free matmuls that keep the PE HAM un-throttled;
                # the target region is cleared by the next start=True matmul
                for _ in range(n):
                    nc.tensor.matmul(
                        pt, wk[:, 0:128], wk[:, 0:128], start=True, stop=True
                    )

            def emit_wo_group(sb, st, mt, warm=0):
                s0 = sb * 512 + st * 128
                msl = slice(mt * 512, (mt + 1) * 512)
                pw = bigp.tile([128, 512], f32, tag="big", name=f"pw{sb}_{st}_{mt}")
                if warm:
                    emit_warm(pw[:, 0:128], warm)
                for kc in range(2):
                    nc.tensor.matmul(
                        pw[:],
                        concatT[kc][:, s0 : s0 + 128],
                        wo[kc][:, msl],
                        start=(kc == 0), stop=(kc == 1),
                    )
                ot = outp.tile([128, 512], bf, tag="ot", name=f"ot{sb}_{st}_{mt}")
                if mt == 0:
                    # alternate ACT/DVE so neither queue backs up behind the
                    # copies (a long DVE queue gates scores-psum recycling)
                    nc.scalar.copy(ot[:], pw[:])
                else:
                    nc.vector.tensor_copy(ot[:], pw[:])
                nc.sync.dma_start(T["out_p"][s0 : s0 + 128, msl], ot[:])

            def emit_norm(sb, pair, po2):
                # po2: 2 psum tiles, each holding 2 s-chunks of [s, 130]
                # (h0 dk0-63 | h0 denom | h1 dk0-63 | h1 denom) at col 0/256.
                # Normalize by the per-partition (per-s) reciprocal denom,
                # write concatN [s, 128j] bf16, PE-transpose to concatT [j, s].
                rc = smallp.tile([128, 8], f32, tag="rc", name=f"rc{sb}_{pair}")
                rcs = smallp.tile([128, 8], f32, tag="rcs", name=f"rcs{sb}_{pair}")
                for sc in range(4):
                    tl, coff = po2[sc // 2], (sc % 2) * 256
                    d2 = tl[:, coff : coff + 130].rearrange(
                        "p (g c) -> p g c", c=65
                    )
                    nc.vector.tensor_copy(rc[:, 2 * sc : 2 * sc + 2],
                                          d2[:, :, 64])
                nc.vector.reciprocal_approx_fast(rcs[:], rc[:])
                s0b = sb * 512
                for sc in range(4):
                    tl, coff = po2[sc // 2], (sc % 2) * 256
                    cn = cnp.tile([128, 128], f32, tag="cn",
                                  name=f"cn{sb}_{pair}_{sc}")
                    for h2 in range(2):
                        src = tl[:, coff + h2 * 65 : coff + h2 * 65 + 64]
                        dst = cn[:, h2 * 64 : h2 * 64 + 64]
                        r1 = rcs[:, 2 * sc + h2 : 2 * sc + h2 + 1]
                        nc.vector.tensor_scalar_mul(dst, src, r1)
                    tp = tpp.tile([128, 128], f32, tag="tp", name=f"tp{sb}_{pair}_{sc}")
                    nc.tensor.transpose(tp[:], cn[:], identf[:])
                    dstT = concatT[pair][:, s0b + sc * 128 : s0b + (sc + 1) * 128]
                    if sc % 2 == 0:
                        nc.scalar.copy(dstT, tp[:])
                    else:
                        nc.vector.tensor_copy(dstT, tp[:])

            emit_mask_dma(0, 0, gated=True)
            emit_vdma(0, gated=True)
            emit_vdma(1, gated=True)
            emit_mask_dma(0, 1, gated=True)
            for i in range(2):
                gate(wo[i][0:1, 0:2])
                nc.gpsimd.dma_start(wo[i][:], T["woT"][i * 128 : (i + 1) * 128, :])
            extras = [
                (lambda s=sbn, j=jt: emit_kproj_jt(s, j))
                for sbn in (1, 2, 3)
                for jt in range(2)
            ]
            po2L = None
            hist = []          # completed pairs: (sb, pair, Pt)
            for sb in range(NSB):
                for pair in range(2):
                    last_it = (sb == NSB - 1 and pair == 1)
                    if sb == 0:
                        extras.append(lambda t=2 * pair: emit_vproj_tb(t))
                        extras.append(lambda t=2 * pair + 1: emit_vproj_tb(t))

                    Pt = ptp.tile(
                        [128, 2 * NC_T * 512], bf, tag="Pt", name=f"Pt{sb}_{pair}"
                    )
                    pv = Pt[:].rearrange("p (c h s) -> p c h s", c=NC_T, h=2)
                    mv = mtiles[sb][:].rearrange("p (c s) -> p c s", c=NC_T)
                    pidx = sb * 2 + pair
                    if pidx >= 1:
                        dr = hist[-1]
                        # attnV out, s-partition orientation: 2 tiles x
                        # 2 s-chunks of [s, 130] at col offsets 0/256
                        po2 = [
                            bigp.tile([128, 512], f32, tag="big",
                                      name=f"av{dr[0]}_{dr[1]}_{i}")
                            for i in range(2)
                        ]

                    def emit_scores_h(ps, c, h2):
                        psl = slice(h2 * 64, h2 * 64 + 64)
                        nc.tensor.matmul(
                            ps[:, h2 * 512 : (h2 + 1) * 512],
                            kpT[pair][c // 8][psl, (c % 8) * 128 : (c % 8 + 1) * 128],
                            qpS[pair][sb][psl, :],
                            start=True, stop=True,
                        )

                    for c in range(NC_T):
                        ps = scp.tile(
                            [128, 1024], f32, tag="sc", name=f"sc{sb}_{pair}_{c}"
                        )
                        # burst-drain groups for this slot (emitted split
                        # around the dense scores matmuls below, which keeps
                        # the PE MAC duty high enough to hold HAM at 8/8)
                        gs = ()
                        psb = ppair = pPt = None
                        if pidx == 1 and c % 2 == 1 and c >= 9:
                            # first drained pair waits for vproj(2)/(3)
                            # (vpc 8-15 land at slots 5/7): bursts at 9-15
                            psb, ppair, pPt = hist[0][:3]
                            gs = (c - 9, c - 8)
                        elif pidx >= 2 and c % 2 == 1:
                            psb, ppair, pPt = hist[-1][:3]
                            gs = (c // 2,)

                        def burst(gidx, ks):
                            sc_, h2_ = gidx // 2, gidx % 2
                            h = ppair * 2 + h2_
                            tl = po2[sc_ // 2]
                            co = (sc_ % 2) * 256 + h2_ * 65
                            for k in ks:
                                pco = (2 * k + h2_) * 512 + sc_ * 128
                                nc.tensor.matmul(
                                    tl[:, co : co + 65],
                                    pPt[:, pco : pco + 128],
                                    vpc[k][:, h * 65 : h * 65 + 65],
                                    start=(k == 0), stop=(k == NC_T - 1),
                                )

                        if pidx >= 1 and c % 2 == 0:
                            # dense dependency-free warm into this slot's
                            # scores psum (the scores start=True discards
                            # it): holds PE MAC duty over the HAM threshold
                            nc.tensor.matmul(
                                ps[:, 0:512], wk[:, 0:128], wk[:, 0:512],
                                start=True, stop=True,
                            )
                        emit_scores_h(ps, c, 0)
                        if gs:
                            burst(gs[0], range(0, 8) if len(gs) == 1
                                  else range(NC_T))
                        emit_scores_h(ps, c, 1)
                        if gs:
                            if len(gs) == 1:
                                burst(gs[0], range(8, NC_T))
                            else:
                                burst(gs[1], range(NC_T))

                        dve_c, pool_c = routes_for(sb)
                        plainL, plainG = _plain_n, PLAIN_GROUPS_N
                        if c in dve_c:
                            # fused exp2 bit-hack + additive mask on DVE:
                            # bits = int16(psum + maskB), reinterpreted bf16.
                            # psum is pre-scaled by ALPHA (folded into kproj);
                            # maskB = 16256 (unmasked) / 1024 (masked -> ~0).
                            for h2 in range(2):
                                nc.vector.tensor_tensor(
                                    Pt[:, (2 * c + h2) * 512 :
                                       (2 * c + h2 + 1) * 512].bitcast(i16),
                                    ps[:, h2 * 512 : (h2 + 1) * 512],
                                    mv[:, c, :], mybir.AluOpType.add,
                                )
                        else:
                            nc.scalar.activation(
                                Pt[:, c * 1024 : (c + 1) * 1024],
                                ps[:], Exp, scale=EXP_SCALE,
                                bias=biasqk[:, 4:5],
                            )
                        if c in pool_c:
                            # mask-mul on the otherwise-idle Pool engine
                            for h2 in range(2):
                                nc.gpsimd.tensor_tensor(
                                    pv[:, c, h2, :], pv[:, c, h2, :],
                                    mv[:, c, :], mybir.AluOpType.mult,
                                )
                        # DVE mask-mul timing for the plain ACT chunks
                        # (DVE_C chunks fold the mask; POOL_C handled above)
                        if last_it and c >= 12:
                            # last iteration: per-chunk mask so attnV can
                            # finish in-loop (no serial tail chain)
                            grp = (c, c + 1) if c in plainL else None
                        else:
                            grp = plainG.get(c)
                        if grp:
                            sl = slice(*grp)
                            for h2 in range(2):
                                nc.vector.tensor_mul(
                                    pv[:, sl, h2, :], pv[:, sl, h2, :],
                                    mv[:, sl, :],
                                )
                        if pidx >= 1 and c == NC_T - 1:
                            nsb_, npair_ = hist[-1][:2]
                            emit_norm(nsb_, npair_, po2)
                            if npair_ == 1:
                                for st in range(4):
                                    for mt in range(2):
                                        extras.append(
                                            lambda s=nsb_, a=st, b=mt:
                                            emit_wo_group(s, a, b)
                                        )
                        if c == 0 and pair == 1 and sb + 1 < NSB:
                            # mask prefetch here (not at pair 0) keeps the
                            # DMA-bound ramp free for the v blocks
                            emit_mask_dma(sb + 1, 0)
                            emit_mask_dma(sb + 1, 1)
                        if c == 1 and pair == 0 and sb + 1 < NSB:
                            emit_qdma(sb + 1)
                        elif c in (1, 3) and pair == 1 and sb + 1 < NSB:
                            emit_qproj_jt(sb + 1, c // 2)
                        elif extras and (
                            (pidx == 0 and c >= 2)
                            or (c % 2 == 1 and (
                                c >= 5 or (pair == 0 and c >= 3)
                                or sb == NSB - 1
                            ))
                        ):
                            extras.pop(0)()
                    hist.append((sb, pair, Pt))
            # tail: burst-drain the last pair, norm, remaining Wo groups
            psb, ppair, pPt = hist[-1]
            po2L = [
                bigp.tile([128, 512], f32, tag="big", name=f"avL_{i}")
                for i in range(2)
            ]
            for gidx in range(8):
                sc, h2 = gidx // 2, gidx % 2
                h = ppair * 2 + h2
                tl = po2L[sc // 2]
                co = (sc % 2) * 256 + h2 * 65
                for k in range(NC_T):
                    pco = (2 * k + h2) * 512 + sc * 128
                    nc.tensor.matmul(
                        tl[:, co : co + 65],
                        pPt[:, pco : pco + 128],
                        vpc[k][:, h * 65 : h * 65 + 65],
                        start=(k == 0), stop=(k == NC_T - 1),
                    )
                if extras:
                    # interleave pair-(3,0) wo groups: dense matmuls keep
                    # the PE duty (HAM) up between small-matmul bursts
                    extras.pop(0)()
            emit_norm(psb, ppair, po2L)
            for fn in extras:
                fn()
            for st in range(4):
                for mt in range(2):
                    emit_wo_group(NSB - 1, st, mt)


def build_nc():
    nc = bacc.Bacc("TRN2", target_bir_lowering=False, debug=False)
    names = {}
    def din(name, shape, dt):
        names[name] = nc.dram_tensor(name, shape, dt, kind="ExternalInput").ap()
    # All inputs pre-packed host-side into exact SBUF tile layout so every
    # DMA is a straight contiguous copy (8-16KB lines, minimal descriptors).
    din("qT", [NSB * 128, 8 * 512], f8)
    din("kT", [NSB * 128, 8 * 512], f8)
    din("vT", [NSB * 2 * 128, 8 * 256], f8)
    din("maskT", [NSB * 128, NC_T * 512], bf)
    din("wqT", [128, 8 * JC], f8)
    din("wkT", [128, 8 * JC], f8)
    din("wvT", [128, 8 * JC], f8)
    din("woT", [JC, D], bf)
    din("biasqk", [128, 5], f32)
    din("ident", [128, 128], f32)
    names["out_p"] = nc.dram_tensor(
        "out_p", [S, D], bf, kind="ExternalOutput"
    ).ap()
    with tile_mod.TileContext(nc) as tc:
        _emit(tc, names)
    nc.compile()
    return nc


_NC = None


def prep_inputs(q, k, v, mask, Wq, bq, Wk, bk, Wv, bv, Wo, bo):
    q = np.asarray(q, F32)
    k = np.asarray(k, F32)
    v = np.asarray(v, F32)
    mask = np.asarray(mask)
    Wq, Wk, Wv, Wo = (np.asarray(w, F32) for w in (Wq, Wk, Wv, Wo))
    bq, bk, bv, bo = (np.asarray(b_, F32) for b_ in (bq, bk, bv, bo))

    def pack_dS(xT, dt):
        # [D, S] -> [NSB*128, 8*512]: [sb*128+p, c*512+s] = xT[c*128+p, sb*512+s]
        x = xT.reshape(8, 128, NSB, 512)
        return np.ascontiguousarray(
            x.transpose(2, 1, 0, 3).reshape(NSB * 128, 8 * 512)
        ).astype(dt)

    def pack_dS_half(xT, dt):
        # t-half granularity: [tb2*128+p, c*256+s] = xT[c*128+p, tb2*256+s]
        x = xT.reshape(8, 128, NSB * 2, 256)
        return np.ascontiguousarray(
            x.transpose(2, 1, 0, 3).reshape(NSB * 2 * 128, 8 * 256)
        ).astype(dt)

    def pack_w(wT):
        # [D, JC] -> [128, 8*JC]: [p, c*JC+j] = wT[c*128+p, j]
        w = wT.reshape(8, 128, JC)
        return np.ascontiguousarray(w.transpose(1, 0, 2).reshape(128, 8 * JC))

    mT0 = mask[0, 0].T  # [t, s]
    m = (mT0 != 0).reshape(NC_T, 128, NSB, 512)
    # Route-dependent mask content per t-chunk c:
    #  c in DVE_C: additive bit-hack bias (16256 unmasked / 1024 masked -> P~0)
    #  else:       multiplicative 1/0
    is_dve = np.isin(np.arange(NC_T), list(DVE_C))[:, None, None, None]
    mvals = np.where(
        is_dve,
        np.where(m, np.float32(16256.0), np.float32(1024.0)),
        m.astype(np.float32),
    )
    maskT = np.ascontiguousarray(
        mvals.transpose(2, 1, 0, 3).reshape(NSB * 128, NC_T * 512)
    ).astype(BF16)
    qT = [pack_dS(q[b_].T, F8) for b_ in range(B)]
    kT = [pack_dS(k[b_].T, F8) for b_ in range(B)]
    vT = [pack_dS_half(v[b_].T, F8) for b_ in range(B)]

    in_maps = []
    for c in range(N_CORES):
        b_, g = c // 4, c % 4
        js = slice(g * JC, (g + 1) * JC)
        # k bias columns carry the ALPHA prefold (kproj scale = WINV*ALPHA);
        # col 4 = ACT_COMP exp bias (pwl route-bias compensation)
        biasqk = np.stack(
            [bq[js][:128], bq[js][128:],
             bk[js][:128] * ALPHA, bk[js][128:] * ALPHA,
             np.full(128, ACT_COMP, F32)], axis=1
        ).astype(F32)
        in_maps.append(
            {
                "qT": qT[b_],
                "kT": kT[b_],
                "vT": vT[b_],
                "maskT": maskT,
                "wqT": pack_w((Wq[js, :].T * WPRE)).astype(F8),
                "wkT": pack_w((Wk[js, :].T * WPRE)).astype(F8),
                "wvT": pack_w((Wv[js, :].T * WPRE)).astype(F8),
                "woT": np.ascontiguousarray(Wo[:, js].T).astype(BF16),
                "biasqk": np.ascontiguousarray(biasqk),
                "ident": np.eye(128, dtype=F32),
            }
        )
    # bv contributes a constant (softmax rows sum to 1): out += Wo @ bv + bo
    bias_out = (Wo @ bv + bo).astype(F32)
    return in_maps, bias_out


def run_prepped(in_maps, bias_out, trace=False, **kw):
    global _NC
    if _NC is None:
        _NC = build_nc()
    res = run_bass_kernel_spmd(
        _NC, in_maps, list(range(N_CORES)), trace=trace, **kw
    )
    out = np.zeros((B, S, D), F32)
    for c in range(N_CORES):
        out[c // 4] += np.asarray(res.results[c]["out_p"], dtype=F32)
    out += bias_out[None, None, :]
    return out, res


def kernel(q, k, v, mask, Wq, bq, Wk, bk, Wv, bv, Wo, bo):
    in_maps, bias_out = prep_inputs(
        q, k, v, mask, Wq, bq, Wk, bk, Wv, bv, Wo, bo
    )
    out, _ = run_prepped(in_maps, bias_out)
    return out



# revision 44
# speedup vs baseline: 1.0112x; 1.0112x over previous
"""Trainium2 Bass kernel for nn_MultiHeadAttention (B=2, S=2048, D=1024, H=16).

Sharding: 8 cores = 2 (batch) x 4 (head groups of 4 heads / 256 dims).
Each core computes QKV projections for its head slice, attention for its 4
heads, and the partial output projection for its 256-dim slice of Wo's input.
Host sums the 4 bf16 partials per batch element (Megatron-style row-parallel
Wo).

All device inputs are pre-packed host-side into exact SBUF tile layout so
every DMA is a straight contiguous copy (8-16KB lines, minimal descriptor
count).  DMA issue order + a data-dependency gate on the second wave
(wv/v/mask/wo behind kproj(0)) keep the round-robin DMA engines focused on
the k/q blocks the head needs first.

Device layouts (per core):
  qT/kT/vT  packed [sb*128+p, c*512+s] bf16
  maskT     packed [sb*128+p, c*512+s] bf16 (0.0/1.0), DMA'd per t-half
  qpT/kpT   [256(j), 2048(s)]   (projections, transposed: j on partitions)
  vp        [2048(t), 4x65]     (natural layout; col 64 of each 65-block = 1.0
                                 -> attn@V matmul also produces softmax denom)
  P~        [t, s] = exp(scoresT/8) * maskT   (scoresT = K_h.T^T @ Q_h.T)
  attn out  [65(j+denom), s] -> normalized -> concatT [256(j), 2048(s)]
  out_p     [2048, 1024] bf16 partial = concatT.T @ woT

Pipeline per (sb, pair) iteration (16 t-chunks): scores (2 concurrent K=64
row-group matmuls) -> exp on ACT (the pacing engine) -> quarter-granular
mask-mul on DVE -> prev iteration's attnV interleaved 1-per-slot (keeps the
PE dense enough to hold HAM at 8/8; per-chunk masks in the last iteration's
final quarter let its attnV finish in-loop).  Norm = fused denominator from
the V ones-column, batched reciprocal + single partition_broadcast;
projection bias-adds and the tail Wo copies ride the ramp/tail-idle ACT.
"""

import sys

import numpy as np

try:
    import concourse.bass as bass
except ImportError:  # pragma: no cover
    sys.path.insert(0, "/opt/trn_rl_repo")
    import concourse.bass as bass

from concourse import bacc

import ml_dtypes

import concourse.tile as tile_mod
from concourse import mybir
from concourse.bass_utils import run_bass_kernel_spmd

BF16 = ml_dtypes.bfloat16
F32 = np.float32

B, S, D, H = 2, 2048, 1024, 16
DK = D // H            # 64
N_CORES = 8
HPC = 4                # heads per core
JC = HPC * DK          # 256 j-dims per core
SCALE = 1.0 / float(np.sqrt(DK))
NSB = S // 512         # 4 s-blocks
NC_T = S // 128        # 16 t-chunks
VROW = HPC * 65        # 260: [h0 64 | 1 | h1 64 | 1 | ...]

bf = mybir.dt.bfloat16
f32 = mybir.dt.float32
f8 = mybir.dt.bfloat16   # fp8 q/k inputs tried and reverted: logit noise 4.5e-2
F8 = BF16
e4 = mybir.dt.float8e4   # fp8 e4m3: OK on scores path only; v-path fp8 costs 4e-2 rel err
E4 = ml_dtypes.float8_e4m3
WPRE = 1.0
WINV = 1.0 / WPRE


def _patch_drain():
    """This walrus build only accepts 1 sync-wait per instruction; the Tile
    exit drain carries one wait per pending proc. Split them across drains."""
    if getattr(tile_mod.TileContext, "_drain_patched", False):
        return
    import bass_rust

    def _drain_and_barrier(self, tick_clock, wait_clock):
        from concourse.tile import ScopedClock

        nc = self.nc
        drain_inst = nc.sync.drain()
        wait_clock.add_sem_waits(
            drain_inst.ins, ScopedClock({None: tick_clock.global_clock})
        )
        si = drain_inst.ins.sync_info
        waits = list(si.on_wait)
        if len(waits) > 1:
            drain_inst.ins.sync_info = bass_rust.SyncInfo(
                on_wait=[waits[0]], on_update=list(si.on_update)
            )
            for w in waits[1:]:
                d2 = nc.sync.drain()
                d2.ins.sync_info = bass_rust.SyncInfo(on_wait=[w], on_update=[])
        nc.all_engine_barrier()
        assert self.sems is not None
        popped = nc._tile_sem_poison_stack.pop()
        assert popped is self._sem_poison
        nc.clear_and_free_semaphores(list(self.sems.allocated().values()))
        nc.all_engine_barrier()

    tile_mod.TileContext._drain_and_barrier = _drain_and_barrier
    tile_mod.TileContext._drain_patched = True


ALPHA = SCALE * float(np.log2(np.e)) * 128.0   # exp2 bit-hack scale, folded
                                               # into the k-projection output
EXP_SCALE = 1.0 / (float(np.log2(np.e)) * 128.0)  # ACT exp scale on the
                                                  # ALPHA-prescaled scores
# ACT-route log-bias matching the bithack's mean pwl overestimate
# E[ln((1+f)/2^f)] = (2ln2-1) - ln2/2: makes the route bias cancel in softmax
ACT_COMP = float(2 * np.log(2) - 1 - np.log(2) / 2)
# Per-iteration chunk routing for the P = exp(scores)*mask production.
# DVE_C: fused on DVE as bits=int16(psum + maskB) reinterpreted bf16
#        (exp2 piecewise-linear bit-hack, mask folded additively).
# POOL_C: ACT exp, then mask-mul on the (otherwise idle) Pool engine.
# Rest:  ACT exp + DVE mask-mul, grouped per contiguous plain run.
DVE_C = (4, 8, 12)
POOL_C = ()


def _mk_groups(dve_c, pool_c):
    plain = [c for c in range(NC_T) if c not in dve_c and c not in pool_c]
    groups, start = {}, None
    for i, c in enumerate(plain):
        if start is None:
            start = c
        if i + 1 >= len(plain) or plain[i + 1] != c + 1:
            groups[c] = (start, c + 1)
            start = None
    return plain, groups


def routes_for(sb):
    return (DVE_C, POOL_C)


_plain_0, PLAIN_GROUPS_0 = _mk_groups((), ())
_plain_n, PLAIN_GROUPS_N = _mk_groups(DVE_C, POOL_C)


def _emit(tc, T):
    nc = tc.nc
    Exp = mybir.ActivationFunctionType.Exp
    i16 = mybir.dt.int16

    from contextlib import ExitStack

    with ExitStack() as ctx:
        persist = ctx.enter_context(tc.tile_pool(name="persist", bufs=1))

        # ---- weights / persistent tiles ----
        # DMA issue order matters: the DMA engines round-robin across active
        # queues, so anything issued early competes with the k/q blocks the
        # head needs first.  Sync queue: wk, wq, biasqk, kT0, qT0, kT1...
        # The wv/wo/mask/v issues are gated behind kproj(0) completion via a
        # dummy gpsimd read of kpT (emitted later).
        wq = persist.tile([128, 8 * JC], f8, tag="wq")
        wk = persist.tile([128, 8 * JC], f8, tag="wk")
        wv = persist.tile([128, 8 * JC], f8, tag="wv")
        for t, name in ((wk, "wkT"), (wq, "wqT")):
            nc.sync.dma_start(t[:], T[name][:, :])
        biasqk = persist.tile([128, 5], f32, tag="biasqk")
        nc.sync.dma_start(biasqk[:], T["biasqk"][:, :])
        identf = persist.tile([128, 128], f32, tag="identf")
        nc.sync.dma_start(identf[:], T["ident"][:, :])
        wo = [persist.tile([128, D], bf, tag=f"wo{i}", name=f"wo{i}") for i in range(2)]

        # per-sb q/k projection tiles ([j, s] transposed layout)
        qpS = [
            [persist.tile([128, 512], bf, tag=f"qp{j}_{s}", name=f"qp{j}_{s}")
             for s in range(NSB)]
            for j in range(2)
        ]
        kpT = [
            [persist.tile([128, 1024], bf, tag=f"kpT{i}_{th}", name=f"kpT{i}_{th}")
             for th in range(2)]
            for i in range(2)
        ]
        # per-chunk v tiles (natural [t, j] layout + ones cols)
        vpc = [persist.tile([128, VROW], bf, tag=f"vp{c}", name=f"vp{c}")
               for c in range(NC_T)]
        concatT = [persist.tile([128, S], bf, tag=f"concatT{i}", name=f"concatT{i}") for i in range(2)]

        wq_v = wq[:].rearrange("p (c j) -> p c j", c=8)
        wk_v = wk[:].rearrange("p (c j) -> p c j", c=8)
        wv_v = wv[:].rearrange("p (c j) -> p c j", c=8)

        q_stream = ctx.enter_context(tc.tile_pool(name="q_stream", bufs=1))
        qtts = {}

        def emit_qdma(sb):
            sl = slice(sb * 512, (sb + 1) * 512)
            qTt = q_stream.tile([128, 8 * 512], f8, tag="qTt", name=f"qTt{sb}")
            nc.sync.dma_start(qTt[:], T["qT"][sb * 128 : (sb + 1) * 128, :])
            qtts[sb] = qTt[:].rearrange("p (c s) -> p c s", c=8)

        def emit_qproj_jt(sb, jt):
            jsl = slice(jt * 128, (jt + 1) * 128)
            ps = bigp.tile([128, 512], f32, tag="big", name=f"pq{sb}_{jt}")
            for c in range(8):
                nc.tensor.matmul(
                    ps[:], wq_v[:, c, jsl], qtts[sb][:, c, :],
                    start=(c == 0), stop=(c == 7),
                )
            # on ACT: keeps the DVE queue short so it never gates scores-psum
            # recycling (the bit-hack chunks consume psum on DVE)
            nc.scalar.activation(
                qpS[jt][sb][:], ps[:], mybir.ActivationFunctionType.Identity,
                bias=biasqk[:, jt : jt + 1], scale=WINV,
            )

        def emit_qproj(sb):
            emit_qdma(sb)
            emit_qproj_jt(sb, 0)
            emit_qproj_jt(sb, 1)

        # ---- attention + output projection ----
        # Chunk-level software pipeline: per t-chunk the PE stream carries
        # scores(i) for both heads (concurrent row-groups), then attnV(i-1)
        # for both heads, plus occasional "extras" (Wo / q-proj / v-proj /
        # mask prefetch). ACT (exp) is the pacing engine; this keeps it fed
        # every chunk while the PE stays dense enough to hold HAM at 8/8.
        if True:
            vstream = ctx.enter_context(tc.tile_pool(name="vstream", bufs=4))
            schp = ctx.enter_context(tc.tile_pool(name="schp", bufs=2))
            maskp = ctx.enter_context(tc.tile_pool(name="maskp", bufs=2))
            ptp = ctx.enter_context(tc.tile_pool(name="ptp", bufs=2))
            smallp = ctx.enter_context(tc.tile_pool(name="smallp", bufs=1))
            cnp = ctx.enter_context(tc.tile_pool(name="cnp", bufs=2))
            outp = ctx.enter_context(tc.tile_pool(name="outp", bufs=4))
            scp = ctx.enter_context(tc.tile_pool(name="scp", bufs=2, space="PSUM"))
            mtiles = {}

        # ---- k/q projections pipelined into the attention loop ----
        # PSUM budget: scp 2x4KB + bigp 3x2KB + tpp 2x256B <= 16KB/partition
        bigp = ctx.enter_context(tc.tile_pool(name="bigp", bufs=3, space="PSUM"))
        tpp = ctx.enter_context(tc.tile_pool(name="tpp", bufs=1, space="PSUM"))
        with tc.tile_pool(name="kv_stream", bufs=2) as kv_stream:
            ktts = {}

            def emit_kdma(sb):
                sl = slice(sb * 512, (sb + 1) * 512)
                kTt = kv_stream.tile([128, 8 * 512], f8, tag="kTt", name=f"kTt{sb}")
                nc.sync.dma_start(kTt[:], T["kT"][sb * 128 : (sb + 1) * 128, :])
                ktts[sb] = kTt[:].rearrange("p (c s) -> p c s", c=8)

            def emit_kproj_jt(sb, jt):
                jsl = slice(jt * 128, (jt + 1) * 128)
                ps = bigp.tile([128, 512], f32, tag="big", name=f"pk{sb}_{jt}")
                for c in range(8):
                    nc.tensor.matmul(
                        ps[:], wk_v[:, c, jsl], ktts[sb][:, c, :],
                        start=(c == 0), stop=(c == 7),
                    )
                # ACT is idle during the ramp where k-proj runs.  ALPHA is
                # folded in here (host pre-scales the k bias columns) so the
                # scores psum arrives pre-scaled for the exp2 bit-hack.
                nc.scalar.activation(
                    kpT[jt][sb // 2][:, (sb % 2) * 512 : (sb % 2 + 1) * 512],
                    ps[:], mybir.ActivationFunctionType.Identity,
                    bias=biasqk[:, 2 + jt : 3 + jt], scale=WINV * ALPHA,
                )

            # Head: get k0/q0 in flight first, project them, then start
            # attention immediately; k1-k3 projections run as extras inside
            # the (0,0) chunk loop (the PE is otherwise idle there since
            # there is no previous attnV to overlap).
            emit_kdma(0)
            emit_qdma(0)
            emit_kdma(1)
            emit_kproj_jt(0, 0)
            emit_kproj_jt(0, 1)
            emit_qproj_jt(0, 0)
            emit_qproj_jt(0, 1)
            emit_kdma(2)
            emit_kdma(3)
            # Gate the second DMA wave (wv/v/mask/wo) behind kproj(0) so the
            # round-robin DMA engines give the head's k/q blocks full
            # bandwidth first.  Tile schedules by data dependency (not
            # program order), so each gated DMA needs a REAL dep: write a
            # corner of its destination tile from kpT first (WAW ordering).
            def gate(dst_corner):
                nc.gpsimd.tensor_copy(dst_corner, kpT[0][0][0:1, 0:2])

            gate(wv[0:1, 0:2])
            nc.gpsimd.dma_start(wv[:], T["wvT"][:, :])




            def emit_mask_dma(sb, half, gated=False):
                # split per t-half so the piece needed by the c==7 mask-mul
                # lands first
                sl = slice(sb * 512, (sb + 1) * 512)
                if half == 0:
                    mT = maskp.tile(
                        [128, NC_T * 512], bf, tag="mT", name=f"mT{sb}"
                    )
                    mtiles[sb] = mT
                mT = mtiles[sb]
                if gated:
                    gate(mT[0:1, half * 4096 : half * 4096 + 2])
                # gated (ramp) issues ride Pool; mid-loop prefetches ride the
                # idle Sync queue so they never sit behind Pool mask-muls
                eng = nc.gpsimd if gated else nc.sync
                eng.dma_start(
                    mT[:, half * 4096 : (half + 1) * 4096],
                    T["maskT"][
                        sb * 128 : (sb + 1) * 128,
                        half * 4096 : (half + 1) * 4096,
                    ],
                )

            vtts = {}

            def emit_vdma(tb, gated=False):
                for hf in range(2):
                    blk = tb * 2 + hf
                    vTt = vstream.tile(
                        [128, 8 * 256], f8, tag="vTt", name=f"vTt{blk}"
                    )
                    if gated:
                        gate(vTt[0:1, 0:2])
                    eng = nc.gpsimd if gated else nc.sync
                    eng.dma_start(
                        vTt[:], T["vT"][blk * 128 : (blk + 1) * 128, :]
                    )
                    vtts[blk] = vTt[:].rearrange("p (c t) -> p c t", c=8)

            def emit_vproj_tb(tb):
                if tb + 2 < NSB and (tb + 2) * 2 not in vtts:
                    emit_vdma(tb + 2)
                for tt in range(4):
                    chunk = tb * 4 + tt
                    ps = bigp.tile([128, 512], f32, tag="big", name=f"pv{chunk}")
                    vTt_v = vtts[tb * 2 + tt // 2]
                    t0 = (tt % 2) * 128
                    for c in range(8):
                        nc.tensor.matmul(
                            ps[:, 0:JC],
                            vTt_v[:, c, t0 : t0 + 128],
                            wv_v[:, c, :],
                            start=(c == 0), stop=(c == 7),
                        )
                    vt = vpc[chunk]
                    nc.gpsimd.memset(
                        vt[:].rearrange("p (h d) -> p h d", d=65)[:, :, 64:65],
                        1.0,
                    )
                    dst = vt[:].rearrange("p (h d) -> p h d", h=HPC)[:, :, 0:DK]
                    src = ps[:, 0:JC].rearrange("p (h d) -> p h d", h=HPC)
                    # ACT is idle during the ramp where vproj runs
                    nc.scalar.mul(dst, src, WINV)

            def emit_warm(pt, n=1):
                # dependency-free matmuls that keep the PE HAM un-throttled;
                # the target region is cleared by the next start=True matmul
                for _ in range(n):
                    nc.tensor.matmul(
                        pt, wk[:, 0:128], wk[:, 0:128], start=True, stop=True
                    )

            def emit_wo_group(sb, st, mt, warm=0):
                s0 = sb * 512 + st * 128
                msl = slice(mt * 512, (mt + 1) * 512)
                pw = bigp.tile([128, 512], f32, tag="big", name=f"pw{sb}_{st}_{mt}")
                if warm:
                    emit_warm(pw[:, 0:128], warm)
                for kc in range(2):
                    nc.tensor.matmul(
                        pw[:],
                        concatT[kc][:, s0 : s0 + 128],
                        wo[kc][:, msl],
                        start=(kc == 0), stop=(kc == 1),
                    )
                ot = outp.tile([128, 512], bf, tag="ot", name=f"ot{sb}_{st}_{mt}")
                if mt == 0:
                    # alternate ACT/DVE so neither queue backs up behind the
                    # copies (a long DVE queue gates scores-psum recycling)
                    nc.scalar.copy(ot[:], pw[:])
                else:
                    nc.vector.tensor_copy(ot[:], pw[:])
                nc.sync.dma_start(T["out_p"][s0 : s0 + 128, msl], ot[:])

            def emit_norm(sb, pair, po2):
                # po2: 2 psum tiles, each holding 2 s-chunks of [s, 130]
                # (h0 dk0-63 | h0 denom | h1 dk0-63 | h1 denom) at col 0/256.
                # Normalize by the per-partition (per-s) reciprocal denom,
                # write concatN [s, 128j] bf16, PE-transpose to concatT [j, s].
                rc = smallp.tile([128, 8], f32, tag="rc", name=f"rc{sb}_{pair}")
                rcs = smallp.tile([128, 8], f32, tag="rcs", name=f"rcs{sb}_{pair}")
                for sc in range(4):
                    tl, coff = po2[sc // 2], (sc % 2) * 256
                    d2 = tl[:, coff : coff + 130].rearrange(
                        "p (g c) -> p g c", c=65
                    )
                    nc.vector.tensor_copy(rc[:, 2 * sc : 2 * sc + 2],
                                          d2[:, :, 64])
                nc.vector.reciprocal_approx_fast(rcs[:], rc[:])
                s0b = sb * 512
                for sc in range(4):
                    tl, coff = po2[sc // 2], (sc % 2) * 256
                    cn = cnp.tile([128, 128], f32, tag="cn",
                                  name=f"cn{sb}_{pair}_{sc}")
                    for h2 in range(2):
                        src = tl[:, coff + h2 * 65 : coff + h2 * 65 + 64]
                        dst = cn[:, h2 * 64 : h2 * 64 + 64]
                        r1 = rcs[:, 2 * sc + h2 : 2 * sc + h2 + 1]
                        nc.vector.tensor_scalar_mul(dst, src, r1)
                    tp = tpp.tile([128, 128], f32, tag="tp", name=f"tp{sb}_{pair}_{sc}")
                    nc.tensor.transpose(tp[:], cn[:], identf[:])
                    dstT = concatT[pair][:, s0b + sc * 128 : s0b + (sc + 1) * 128]
                    if sc % 2 == 0:
                        nc.scalar.copy(dstT, tp[:])
                    else:
                        nc.vector.tensor_copy(dstT, tp[:])

            emit_mask_dma(0, 0, gated=True)
            emit_vdma(0, gated=True)
            emit_vdma(1, gated=True)
            emit_mask_dma(0, 1, gated=True)
            for i in range(2):
                gate(wo[i][0:1, 0:2])
                nc.gpsimd.dma_start(wo[i][:], T["woT"][i * 128 : (i + 1) * 128, :])
            extras = [
                (lambda s=sbn, j=jt: emit_kproj_jt(s, j))
                for sbn in (1, 2, 3)
                for jt in range(2)
            ]
            po2L = None
            hist = []          # completed pairs: (sb, pair, Pt)
            for sb in range(NSB):
                for pair in range(2):
                    last_it = (sb == NSB - 1 and pair == 1)
                    if sb == 0:
                        extras.append(lambda t=2 * pair: emit_vproj_tb(t))
                        extras.append(lambda t=2 * pair + 1: emit_vproj_tb(t))

                    Pt = ptp.tile(
                        [128, 2 * NC_T * 512], bf, tag="Pt", name=f"Pt{sb}_{pair}"
                    )
                    pv = Pt[:].rearrange("p (c h s) -> p c h s", c=NC_T, h=2)
                    mv = mtiles[sb][:].rearrange("p (c s) -> p c s", c=NC_T)
                    pidx = sb * 2 + pair
                    if pidx >= 1:
                        dr = hist[-1]
                        # attnV out, s-partition orientation: 2 tiles x
                        # 2 s-chunks of [s, 130] at col offsets 0/256
                        po2 = [
                            bigp.tile([128, 512], f32, tag="big",
                                      name=f"av{dr[0]}_{dr[1]}_{i}")
                            for i in range(2)
                        ]

                    def emit_scores_h(ps, c, h2):
                        psl = slice(h2 * 64, h2 * 64 + 64)
                        nc.tensor.matmul(
                            ps[:, h2 * 512 : (h2 + 1) * 512],
                            kpT[pair][c // 8][psl, (c % 8) * 128 : (c % 8 + 1) * 128],
                            qpS[pair][sb][psl, :],
                            start=True, stop=True,
                        )

                    for c in range(NC_T):
                        ps = scp.tile(
                            [128, 1024], f32, tag="sc", name=f"sc{sb}_{pair}_{c}"
                        )
                        # burst-drain groups for this slot (emitted split
                        # around the dense scores matmuls below, which keeps
                        # the PE MAC duty high enough to hold HAM at 8/8)
                        gs = ()
                        psb = ppair = pPt = None
                        if pidx == 1 and c % 2 == 1 and c >= 9:
                            # first drained pair waits for vproj(2)/(3)
                            # (vpc 8-15 land at slots 5/7): bursts at 9-15
                            psb, ppair, pPt = hist[0][:3]
                            gs = (c - 9, c - 8)
                        elif pidx >= 2 and c % 2 == 1:
                            psb, ppair, pPt = hist[-1][:3]
                            gs = (c // 2,)

                        def burst(gidx, ks):
                            sc_, h2_ = gidx // 2, gidx % 2
                            h = ppair * 2 + h2_
                            tl = po2[sc_ // 2]
                            co = (sc_ % 2) * 256 + h2_ * 65
                            for k in ks:
                                pco = (2 * k + h2_) * 512 + sc_ * 128
                                nc.tensor.matmul(
                                    tl[:, co : co + 65],
                                    pPt[:, pco : pco + 128],
                                    vpc[k][:, h * 65 : h * 65 + 65],
                                    start=(k == 0), stop=(k == NC_T - 1),
                                )

                        emit_scores_h(ps, c, 0)
                        if gs:
                            burst(gs[0], range(0, 8) if len(gs) == 1
                                  else range(NC_T))
                        emit_scores_h(ps, c, 1)
                        if gs:
                            if len(gs) == 1:
                                burst(gs[0], range(8, NC_T))
                            else:
                                burst(gs[1], range(NC_T))

                        dve_c, pool_c = routes_for(sb)
                        plainL, plainG = _plain_n, PLAIN_GROUPS_N
                        if c in dve_c:
                            # fused exp2 bit-hack + additive mask on DVE:
                            # bits = int16(psum + maskB), reinterpreted bf16.
                            # psum is pre-scaled by ALPHA (folded into kproj);
                            # maskB = 16256 (unmasked) / 1024 (masked -> ~0).
                            for h2 in range(2):
                                nc.vector.tensor_tensor(
                                    Pt[:, (2 * c + h2) * 512 :
                                       (2 * c + h2 + 1) * 512].bitcast(i16),
                                    ps[:, h2 * 512 : (h2 + 1) * 512],
                                    mv[:, c, :], mybir.AluOpType.add,
                                )
                        else:
                            nc.scalar.activation(
                                Pt[:, c * 1024 : (c + 1) * 1024],
                                ps[:], Exp, scale=EXP_SCALE,
                                bias=biasqk[:, 4:5],
                            )
                        if c in pool_c:
                            # mask-mul on the otherwise-idle Pool engine
                            for h2 in range(2):
                                nc.gpsimd.tensor_tensor(
                                    pv[:, c, h2, :], pv[:, c, h2, :],
                                    mv[:, c, :], mybir.AluOpType.mult,
                                )
                        # DVE mask-mul timing for the plain ACT chunks
                        # (DVE_C chunks fold the mask; POOL_C handled above)
                        if last_it and c >= 12:
                            # last iteration: per-chunk mask so attnV can
                            # finish in-loop (no serial tail chain)
                            grp = (c, c + 1) if c in plainL else None
                        else:
                            grp = plainG.get(c)
                        if grp:
                            sl = slice(*grp)
                            for h2 in range(2):
                                nc.vector.tensor_mul(
                                    pv[:, sl, h2, :], pv[:, sl, h2, :],
                                    mv[:, sl, :],
                                )
                        if pidx >= 1 and c == NC_T - 1:
                            nsb_, npair_ = hist[-1][:2]
                            emit_norm(nsb_, npair_, po2)
                            if npair_ == 1:
                                for st in range(4):
                                    for mt in range(2):
                                        extras.append(
                                            lambda s=nsb_, a=st, b=mt:
                                            emit_wo_group(s, a, b)
                                        )
                        if c == 0 and pair == 1 and sb + 1 < NSB:
                            # mask prefetch here (not at pair 0) keeps the
                            # DMA-bound ramp free for the v blocks
                            emit_mask_dma(sb + 1, 0)
                            emit_mask_dma(sb + 1, 1)
                        if c == 1 and pair == 0 and sb + 1 < NSB:
                            emit_qdma(sb + 1)
                        elif c in (1, 3) and pair == 1 and sb + 1 < NSB:
                            emit_qproj_jt(sb + 1, c // 2)
                        elif extras and (
                            (pidx == 0 and c >= 2)
                            or (c % 2 == 1 and (
                                c >= 5 or (pair == 0 and c >= 3)
                                or sb == NSB - 1
                            ))
                        ):
                            extras.pop(0)()
                    hist.append((sb, pair, Pt))
            # tail: burst-drain the last pair, norm, remaining Wo groups
            psb, ppair, pPt = hist[-1]
            po2L = [
                bigp.tile([128, 512], f32, tag="big", name=f"avL_{i}")
                for i in range(2)
            ]
            for gidx in range(8):
                sc, h2 = gidx // 2, gidx % 2
                h = ppair * 2 + h2
                tl = po2L[sc // 2]
                co = (sc % 2) * 256 + h2 * 65
                for k in range(NC_T):
                    pco = (2 * k + h2) * 512 + sc * 128
                    nc.tensor.matmul(
                        tl[:, co : co + 65],
                        pPt[:, pco : pco + 128],
                        vpc[k][:, h * 65 : h * 65 + 65],
                        start=(k == 0), stop=(k == NC_T - 1),
                    )
                if extras:
                    # interleave pair-(3,0) wo groups: dense matmuls keep
                    # the PE duty (HAM) up between small-matmul bursts
                    extras.pop(0)()
            emit_norm(psb, ppair, po2L)
            for fn in extras:
                fn()
            for st in range(4):
                for mt in range(2):
                    emit_wo_group(NSB - 1, st, mt)


def build_nc():
    nc = bacc.Bacc("TRN2", target_bir_lowering=False, debug=False)
    names = {}
    def din(name, shape, dt):
        names[name] = nc.dram_tensor(name, shape, dt, kind="ExternalInput").ap()
    # All inputs pre-packed host-side into exact SBUF tile layout so every
    # DMA is a straight contiguous copy (8-16KB lines, minimal descriptors).
    din("qT", [NSB * 128, 8 * 512], f8)
    din("kT", [NSB * 128, 8 * 512], f8)
    din("vT", [NSB * 2 * 128, 8 * 256], f8)
    din("maskT", [NSB * 128, NC_T * 512], bf)
    din("wqT", [128, 8 * JC], f8)
    din("wkT", [128, 8 * JC], f8)
    din("wvT", [128, 8 * JC], f8)
    din("woT", [JC, D], bf)
    din("biasqk", [128, 5], f32)
    din("ident", [128, 128], f32)
    names["out_p"] = nc.dram_tensor(
        "out_p", [S, D], bf, kind="ExternalOutput"
    ).ap()
    with tile_mod.TileContext(nc) as tc:
        _emit(tc, names)
    nc.compile()
    return nc


_NC = None


def prep_inputs(q, k, v, mask, Wq, bq, Wk, bk, Wv, bv, Wo, bo):
    q = np.asarray(q, F32)
    k = np.asarray(k, F32)
    v = np.asarray(v, F32)
    mask = np.asarray(mask)
    Wq, Wk, Wv, Wo = (np.asarray(w, F32) for w in (Wq, Wk, Wv, Wo))
    bq, bk, bv, bo = (np.asarray(b_, F32) for b_ in (bq, bk, bv, bo))

    def pack_dS(xT, dt):
        # [D, S] -> [NSB*128, 8*512]: [sb*128+p, c*512+s] = xT[c*128+p, sb*512+s]
        x = xT.reshape(8, 128, NSB, 512)
        return np.ascontiguousarray(
            x.transpose(2, 1, 0, 3).reshape(NSB * 128, 8 * 512)
        ).astype(dt)

    def pack_dS_half(xT, dt):
        # t-half granularity: [tb2*128+p, c*256+s] = xT[c*128+p, tb2*256+s]
        x = xT.reshape(8, 128, NSB * 2, 256)
        return np.ascontiguousarray(
            x.transpose(2, 1, 0, 3).reshape(NSB * 2 * 128, 8 * 256)
        ).astype(dt)

    def pack_w(wT):
        # [D, JC] -> [128, 8*JC]: [p, c*JC+j] = wT[c*128+p, j]
        w = wT.reshape(8, 128, JC)
        return np.ascontiguousarray(w.transpose(1, 0, 2).reshape(128, 8 * JC))

    mT0 = mask[0, 0].T  # [t, s]
    m = (mT0 != 0).reshape(NC_T, 128, NSB, 512)
    # Route-dependent mask content per t-chunk c:
    #  c in DVE_C: additive bit-hack bias (16256 unmasked / 1024 masked -> P~0)
    #  else:       multiplicative 1/0
    is_dve = np.isin(np.arange(NC_T), list(DVE_C))[:, None, None, None]
    mvals = np.where(
        is_dve,
        np.where(m, np.float32(16256.0), np.float32(1024.0)),
        m.astype(np.float32),
    )
    maskT = np.ascontiguousarray(
        mvals.transpose(2, 1, 0, 3).reshape(NSB * 128, NC_T * 512)
    ).astype(BF16)
    qT = [pack_dS(q[b_].T, F8) for b_ in range(B)]
    kT = [pack_dS(k[b_].T, F8) for b_ in range(B)]
    vT = [pack_dS_half(v[b_].T, F8) for b_ in range(B)]

    in_maps = []
    for c in range(N_CORES):
        b_, g = c // 4, c % 4
        js = slice(g * JC, (g + 1) * JC)
        # k bias columns carry the ALPHA prefold (kproj scale = WINV*ALPHA);
        # col 4 = ACT_COMP exp bias (pwl route-bias compensation)
        biasqk = np.stack(
            [bq[js][:128], bq[js][128:],
             bk[js][:128] * ALPHA, bk[js][128:] * ALPHA,
             np.full(128, ACT_COMP, F32)], axis=1
        ).astype(F32)
        in_maps.append(
            {
                "qT": qT[b_],
                "kT": kT[b_],
                "vT": vT[b_],
                "maskT": maskT,
                "wqT": pack_w((Wq[js, :].T * WPRE)).astype(F8),
                "wkT": pack_w((Wk[js, :].T * WPRE)).astype(F8),
                "wvT": pack_w((Wv[js, :].T * WPRE)).astype(F8),
                "woT": np.ascontiguousarray(Wo[:, js].T).astype(BF16),
                "biasqk": np.ascontiguousarray(biasqk),
                "ident": np.eye(128, dtype=F32),
            }
        )
    # bv contributes a constant (softmax rows sum to 1): out += Wo @ bv + bo
    bias_out = (Wo @ bv + bo).astype(F32)
    return in_maps, bias_out


def run_prepped(in_maps, bias_out, trace=False, **kw):
    global _NC
    if _NC is None:
        _NC = build_nc()
    res = run_bass_kernel_spmd(
        _NC, in_maps, list(range(N_CORES)), trace=trace, **kw
    )
    out = np.zeros((B, S, D), F32)
    for c in range(N_CORES):
        out[c // 4] += np.asarray(res.results[c]["out_p"], dtype=F32)
    out += bias_out[None, None, :]
    return out, res


def kernel(q, k, v, mask, Wq, bq, Wk, bk, Wv, bv, Wo, bo):
    in_maps, bias_out = prep_inputs(
        q, k, v, mask, Wq, bq, Wk, bk, Wv, bv, Wo, bo
    )
    out, _ = run_prepped(in_maps, bias_out)
    return out



# revision 45
# speedup vs baseline: 1.0325x; 1.0211x over previous
"""Trainium2 Bass kernel for nn_MultiHeadAttention (B=2, S=2048, D=1024, H=16).

Sharding: 8 cores = 2 (batch) x 4 (head groups of 4 heads / 256 dims).
Each core computes QKV projections for its head slice, attention for its 4
heads, and the partial output projection for its 256-dim slice of Wo's input.
Host sums the 4 bf16 partials per batch element (Megatron-style row-parallel
Wo).

All device inputs are pre-packed host-side into exact SBUF tile layout so
every DMA is a straight contiguous copy (8-16KB lines, minimal descriptor
count).  DMA issue order + a data-dependency gate on the second wave
(wv/v/mask/wo behind kproj(0)) keep the round-robin DMA engines focused on
the k/q blocks the head needs first.

Device layouts (per core):
  qT/kT/vT  packed [sb*128+p, c*512+s] bf16
  maskT     packed [sb*128+p, c*512+s] bf16 (0.0/1.0), DMA'd per t-half
  qpT/kpT   [256(j), 2048(s)]   (projections, transposed: j on partitions)
  vp        [2048(t), 4x65]     (natural layout; col 64 of each 65-block = 1.0
                                 -> attn@V matmul also produces softmax denom)
  P~        [t, s] = exp(scoresT/8) * maskT   (scoresT = K_h.T^T @ Q_h.T)
  attn out  [65(j+denom), s] -> normalized -> concatT [256(j), 2048(s)]
  out_p     [2048, 1024] bf16 partial = concatT.T @ woT

Pipeline per (sb, pair) iteration (16 t-chunks): scores (2 concurrent K=64
row-group matmuls) -> exp on ACT (the pacing engine) -> quarter-granular
mask-mul on DVE -> prev iteration's attnV interleaved 1-per-slot (keeps the
PE dense enough to hold HAM at 8/8; per-chunk masks in the last iteration's
final quarter let its attnV finish in-loop).  Norm = fused denominator from
the V ones-column, batched reciprocal + single partition_broadcast;
projection bias-adds and the tail Wo copies ride the ramp/tail-idle ACT.
"""

import sys

import numpy as np

try:
    import concourse.bass as bass
except ImportError:  # pragma: no cover
    sys.path.insert(0, "/opt/trn_rl_repo")
    import concourse.bass as bass

from concourse import bacc

import ml_dtypes

import concourse.tile as tile_mod
from concourse import mybir
from concourse.bass_utils import run_bass_kernel_spmd

BF16 = ml_dtypes.bfloat16
F32 = np.float32

B, S, D, H = 2, 2048, 1024, 16
DK = D // H            # 64
N_CORES = 8
HPC = 4                # heads per core
JC = HPC * DK          # 256 j-dims per core
SCALE = 1.0 / float(np.sqrt(DK))
NSB = S // 512         # 4 s-blocks
NC_T = S // 128        # 16 t-chunks
VROW = HPC * 65        # 260: [h0 64 | 1 | h1 64 | 1 | ...]

bf = mybir.dt.bfloat16
f32 = mybir.dt.float32
f8 = mybir.dt.bfloat16   # fp8 q/k inputs tried and reverted: logit noise 4.5e-2
F8 = BF16
e4 = mybir.dt.float8e4   # fp8 e4m3: OK on scores path only; v-path fp8 costs 4e-2 rel err
E4 = ml_dtypes.float8_e4m3
WPRE = 1.0
WINV = 1.0 / WPRE


def _patch_drain():
    """This walrus build only accepts 1 sync-wait per instruction; the Tile
    exit drain carries one wait per pending proc. Split them across drains."""
    if getattr(tile_mod.TileContext, "_drain_patched", False):
        return
    import bass_rust

    def _drain_and_barrier(self, tick_clock, wait_clock):
        from concourse.tile import ScopedClock

        nc = self.nc
        drain_inst = nc.sync.drain()
        wait_clock.add_sem_waits(
            drain_inst.ins, ScopedClock({None: tick_clock.global_clock})
        )
        si = drain_inst.ins.sync_info
        waits = list(si.on_wait)
        if len(waits) > 1:
            drain_inst.ins.sync_info = bass_rust.SyncInfo(
                on_wait=[waits[0]], on_update=list(si.on_update)
            )
            for w in waits[1:]:
                d2 = nc.sync.drain()
                d2.ins.sync_info = bass_rust.SyncInfo(on_wait=[w], on_update=[])
        nc.all_engine_barrier()
        assert self.sems is not None
        popped = nc._tile_sem_poison_stack.pop()
        assert popped is self._sem_poison
        nc.clear_and_free_semaphores(list(self.sems.allocated().values()))
        nc.all_engine_barrier()

    tile_mod.TileContext._drain_and_barrier = _drain_and_barrier
    tile_mod.TileContext._drain_patched = True


ALPHA = SCALE * float(np.log2(np.e)) * 128.0   # exp2 bit-hack scale, folded
                                               # into the k-projection output
EXP_SCALE = 1.0 / (float(np.log2(np.e)) * 128.0)  # ACT exp scale on the
                                                  # ALPHA-prescaled scores
# ACT-route log-bias matching the bithack's mean pwl overestimate
# E[ln((1+f)/2^f)] = (2ln2-1) - ln2/2: makes the route bias cancel in softmax
ACT_COMP = float(2 * np.log(2) - 1 - np.log(2) / 2)
# Per-iteration chunk routing for the P = exp(scores)*mask production.
# DVE_C: fused on DVE as bits=int16(psum + maskB) reinterpreted bf16
#        (exp2 piecewise-linear bit-hack, mask folded additively).
# POOL_C: ACT exp, then mask-mul on the (otherwise idle) Pool engine.
# Rest:  ACT exp + DVE mask-mul, grouped per contiguous plain run.
DVE_C = (2, 5, 8, 11, 14)
POOL_C = ()


def _mk_groups(dve_c, pool_c):
    plain = [c for c in range(NC_T) if c not in dve_c and c not in pool_c]
    groups, start = {}, None
    for i, c in enumerate(plain):
        if start is None:
            start = c
        if i + 1 >= len(plain) or plain[i + 1] != c + 1:
            groups[c] = (start, c + 1)
            start = None
    return plain, groups


def routes_for(sb):
    return (DVE_C, POOL_C)


_plain_0, PLAIN_GROUPS_0 = _mk_groups((), ())
_plain_n, PLAIN_GROUPS_N = _mk_groups(DVE_C, POOL_C)


def _emit(tc, T):
    nc = tc.nc
    Exp = mybir.ActivationFunctionType.Exp
    i16 = mybir.dt.int16

    from contextlib import ExitStack

    with ExitStack() as ctx:
        persist = ctx.enter_context(tc.tile_pool(name="persist", bufs=1))

        # ---- weights / persistent tiles ----
        # DMA issue order matters: the DMA engines round-robin across active
        # queues, so anything issued early competes with the k/q blocks the
        # head needs first.  Sync queue: wk, wq, biasqk, kT0, qT0, kT1...
        # The wv/wo/mask/v issues are gated behind kproj(0) completion via a
        # dummy gpsimd read of kpT (emitted later).
        wq = persist.tile([128, 8 * JC], f8, tag="wq")
        wk = persist.tile([128, 8 * JC], f8, tag="wk")
        wv = persist.tile([128, 8 * JC], f8, tag="wv")
        for t, name in ((wk, "wkT"), (wq, "wqT")):
            nc.sync.dma_start(t[:], T[name][:, :])
        biasqk = persist.tile([128, 5], f32, tag="biasqk")
        nc.sync.dma_start(biasqk[:], T["biasqk"][:, :])
        identf = persist.tile([128, 128], f32, tag="identf")
        nc.sync.dma_start(identf[:], T["ident"][:, :])
        wo = [persist.tile([128, D], bf, tag=f"wo{i}", name=f"wo{i}") for i in range(2)]

        # per-sb q/k projection tiles ([j, s] transposed layout)
        qpS = [
            [persist.tile([128, 512], bf, tag=f"qp{j}_{s}", name=f"qp{j}_{s}")
             for s in range(NSB)]
            for j in range(2)
        ]
        kpT = [
            [persist.tile([128, 1024], bf, tag=f"kpT{i}_{th}", name=f"kpT{i}_{th}")
             for th in range(2)]
            for i in range(2)
        ]
        # per-chunk v tiles (natural [t, j] layout + ones cols)
        vpc = [persist.tile([128, VROW], bf, tag=f"vp{c}", name=f"vp{c}")
               for c in range(NC_T)]
        concatT = [persist.tile([128, S], bf, tag=f"concatT{i}", name=f"concatT{i}") for i in range(2)]

        wq_v = wq[:].rearrange("p (c j) -> p c j", c=8)
        wk_v = wk[:].rearrange("p (c j) -> p c j", c=8)
        wv_v = wv[:].rearrange("p (c j) -> p c j", c=8)

        q_stream = ctx.enter_context(tc.tile_pool(name="q_stream", bufs=1))
        qtts = {}

        def emit_qdma(sb):
            sl = slice(sb * 512, (sb + 1) * 512)
            qTt = q_stream.tile([128, 8 * 512], f8, tag="qTt", name=f"qTt{sb}")
            nc.sync.dma_start(qTt[:], T["qT"][sb * 128 : (sb + 1) * 128, :])
            qtts[sb] = qTt[:].rearrange("p (c s) -> p c s", c=8)

        def emit_qproj_jt(sb, jt):
            jsl = slice(jt * 128, (jt + 1) * 128)
            ps = bigp.tile([128, 512], f32, tag="big", name=f"pq{sb}_{jt}")
            for c in range(8):
                nc.tensor.matmul(
                    ps[:], wq_v[:, c, jsl], qtts[sb][:, c, :],
                    start=(c == 0), stop=(c == 7),
                )
            # on ACT: keeps the DVE queue short so it never gates scores-psum
            # recycling (the bit-hack chunks consume psum on DVE)
            nc.scalar.activation(
                qpS[jt][sb][:], ps[:], mybir.ActivationFunctionType.Identity,
                bias=biasqk[:, jt : jt + 1], scale=WINV,
            )

        def emit_qproj(sb):
            emit_qdma(sb)
            emit_qproj_jt(sb, 0)
            emit_qproj_jt(sb, 1)

        # ---- attention + output projection ----
        # Chunk-level software pipeline: per t-chunk the PE stream carries
        # scores(i) for both heads (concurrent row-groups), then attnV(i-1)
        # for both heads, plus occasional "extras" (Wo / q-proj / v-proj /
        # mask prefetch). ACT (exp) is the pacing engine; this keeps it fed
        # every chunk while the PE stays dense enough to hold HAM at 8/8.
        if True:
            vstream = ctx.enter_context(tc.tile_pool(name="vstream", bufs=4))
            schp = ctx.enter_context(tc.tile_pool(name="schp", bufs=2))
            maskp = ctx.enter_context(tc.tile_pool(name="maskp", bufs=2))
            ptp = ctx.enter_context(tc.tile_pool(name="ptp", bufs=2))
            smallp = ctx.enter_context(tc.tile_pool(name="smallp", bufs=1))
            cnp = ctx.enter_context(tc.tile_pool(name="cnp", bufs=2))
            outp = ctx.enter_context(tc.tile_pool(name="outp", bufs=4))
            scp = ctx.enter_context(tc.tile_pool(name="scp", bufs=2, space="PSUM"))
            mtiles = {}

        # ---- k/q projections pipelined into the attention loop ----
        # PSUM budget: scp 2x4KB + bigp 3x2KB + tpp 2x256B <= 16KB/partition
        bigp = ctx.enter_context(tc.tile_pool(name="bigp", bufs=3, space="PSUM"))
        tpp = ctx.enter_context(tc.tile_pool(name="tpp", bufs=1, space="PSUM"))
        with tc.tile_pool(name="kv_stream", bufs=2) as kv_stream:
            ktts = {}

            def emit_kdma(sb):
                sl = slice(sb * 512, (sb + 1) * 512)
                kTt = kv_stream.tile([128, 8 * 512], f8, tag="kTt", name=f"kTt{sb}")
                nc.sync.dma_start(kTt[:], T["kT"][sb * 128 : (sb + 1) * 128, :])
                ktts[sb] = kTt[:].rearrange("p (c s) -> p c s", c=8)

            def emit_kproj_jt(sb, jt):
                jsl = slice(jt * 128, (jt + 1) * 128)
                ps = bigp.tile([128, 512], f32, tag="big", name=f"pk{sb}_{jt}")
                for c in range(8):
                    nc.tensor.matmul(
                        ps[:], wk_v[:, c, jsl], ktts[sb][:, c, :],
                        start=(c == 0), stop=(c == 7),
                    )
                # ACT is idle during the ramp where k-proj runs.  ALPHA is
                # folded in here (host pre-scales the k bias columns) so the
                # scores psum arrives pre-scaled for the exp2 bit-hack.
                nc.scalar.activation(
                    kpT[jt][sb // 2][:, (sb % 2) * 512 : (sb % 2 + 1) * 512],
                    ps[:], mybir.ActivationFunctionType.Identity,
                    bias=biasqk[:, 2 + jt : 3 + jt], scale=WINV * ALPHA,
                )

            # Head: get k0/q0 in flight first, project them, then start
            # attention immediately; k1-k3 projections run as extras inside
            # the (0,0) chunk loop (the PE is otherwise idle there since
            # there is no previous attnV to overlap).
            emit_kdma(0)
            emit_qdma(0)
            emit_kdma(1)
            emit_kproj_jt(0, 0)
            emit_kproj_jt(0, 1)
            emit_qproj_jt(0, 0)
            emit_qproj_jt(0, 1)
            emit_kdma(2)
            emit_kdma(3)
            # Gate the second DMA wave (wv/v/mask/wo) behind kproj(0) so the
            # round-robin DMA engines give the head's k/q blocks full
            # bandwidth first.  Tile schedules by data dependency (not
            # program order), so each gated DMA needs a REAL dep: write a
            # corner of its destination tile from kpT first (WAW ordering).
            def gate(dst_corner):
                nc.gpsimd.tensor_copy(dst_corner, kpT[0][0][0:1, 0:2])

            gate(wv[0:1, 0:2])
            nc.gpsimd.dma_start(wv[:], T["wvT"][:, :])




            def emit_mask_dma(sb, half, gated=False):
                # split per t-half so the piece needed by the c==7 mask-mul
                # lands first
                sl = slice(sb * 512, (sb + 1) * 512)
                if half == 0:
                    mT = maskp.tile(
                        [128, NC_T * 512], bf, tag="mT", name=f"mT{sb}"
                    )
                    mtiles[sb] = mT
                mT = mtiles[sb]
                if gated:
                    gate(mT[0:1, half * 4096 : half * 4096 + 2])
                # gated (ramp) issues ride Pool; mid-loop prefetches ride the
                # idle Sync queue so they never sit behind Pool mask-muls
                eng = nc.gpsimd if gated else nc.sync
                eng.dma_start(
                    mT[:, half * 4096 : (half + 1) * 4096],
                    T["maskT"][
                        sb * 128 : (sb + 1) * 128,
                        half * 4096 : (half + 1) * 4096,
                    ],
                )

            vtts = {}

            def emit_vdma(tb, gated=False):
                for hf in range(2):
                    blk = tb * 2 + hf
                    vTt = vstream.tile(
                        [128, 8 * 256], f8, tag="vTt", name=f"vTt{blk}"
                    )
                    if gated:
                        gate(vTt[0:1, 0:2])
                    eng = nc.gpsimd if gated else nc.sync
                    eng.dma_start(
                        vTt[:], T["vT"][blk * 128 : (blk + 1) * 128, :]
                    )
                    vtts[blk] = vTt[:].rearrange("p (c t) -> p c t", c=8)

            def emit_vproj_tb(tb):
                if tb + 2 < NSB and (tb + 2) * 2 not in vtts:
                    emit_vdma(tb + 2)
                for tt in range(4):
                    chunk = tb * 4 + tt
                    ps = bigp.tile([128, 512], f32, tag="big", name=f"pv{chunk}")
                    vTt_v = vtts[tb * 2 + tt // 2]
                    t0 = (tt % 2) * 128
                    for c in range(8):
                        nc.tensor.matmul(
                            ps[:, 0:JC],
                            vTt_v[:, c, t0 : t0 + 128],
                            wv_v[:, c, :],
                            start=(c == 0), stop=(c == 7),
                        )
                    vt = vpc[chunk]
                    nc.gpsimd.memset(
                        vt[:].rearrange("p (h d) -> p h d", d=65)[:, :, 64:65],
                        1.0,
                    )
                    dst = vt[:].rearrange("p (h d) -> p h d", h=HPC)[:, :, 0:DK]
                    src = ps[:, 0:JC].rearrange("p (h d) -> p h d", h=HPC)
                    # ACT is idle during the ramp where vproj runs
                    nc.scalar.mul(dst, src, WINV)

            def emit_warm(pt, n=1):
                # dependency-free matmuls that keep the PE HAM un-throttled;
                # the target region is cleared by the next start=True matmul
                for _ in range(n):
                    nc.tensor.matmul(
                        pt, wk[:, 0:128], wk[:, 0:128], start=True, stop=True
                    )

            def emit_wo_group(sb, st, mt, warm=0):
                s0 = sb * 512 + st * 128
                msl = slice(mt * 512, (mt + 1) * 512)
                pw = bigp.tile([128, 512], f32, tag="big", name=f"pw{sb}_{st}_{mt}")
                if warm:
                    emit_warm(pw[:, 0:128], warm)
                for kc in range(2):
                    nc.tensor.matmul(
                        pw[:],
                        concatT[kc][:, s0 : s0 + 128],
                        wo[kc][:, msl],
                        start=(kc == 0), stop=(kc == 1),
                    )
                ot = outp.tile([128, 512], bf, tag="ot", name=f"ot{sb}_{st}_{mt}")
                if mt == 0:
                    # alternate ACT/DVE so neither queue backs up behind the
                    # copies (a long DVE queue gates scores-psum recycling)
                    nc.scalar.copy(ot[:], pw[:])
                else:
                    nc.vector.tensor_copy(ot[:], pw[:])
                nc.sync.dma_start(T["out_p"][s0 : s0 + 128, msl], ot[:])

            def emit_norm(sb, pair, po2):
                # po2: 2 psum tiles, each holding 2 s-chunks of [s, 130]
                # (h0 dk0-63 | h0 denom | h1 dk0-63 | h1 denom) at col 0/256.
                # Normalize by the per-partition (per-s) reciprocal denom,
                # write concatN [s, 128j] bf16, PE-transpose to concatT [j, s].
                rc = smallp.tile([128, 8], f32, tag="rc", name=f"rc{sb}_{pair}")
                rcs = smallp.tile([128, 8], f32, tag="rcs", name=f"rcs{sb}_{pair}")
                for sc in range(4):
                    tl, coff = po2[sc // 2], (sc % 2) * 256
                    d2 = tl[:, coff : coff + 130].rearrange(
                        "p (g c) -> p g c", c=65
                    )
                    nc.vector.tensor_copy(rc[:, 2 * sc : 2 * sc + 2],
                                          d2[:, :, 64])
                nc.vector.reciprocal_approx_fast(rcs[:], rc[:])
                s0b = sb * 512
                for sc in range(4):
                    tl, coff = po2[sc // 2], (sc % 2) * 256
                    cn = cnp.tile([128, 128], f32, tag="cn",
                                  name=f"cn{sb}_{pair}_{sc}")
                    for h2 in range(2):
                        src = tl[:, coff + h2 * 65 : coff + h2 * 65 + 64]
                        dst = cn[:, h2 * 64 : h2 * 64 + 64]
                        r1 = rcs[:, 2 * sc + h2 : 2 * sc + h2 + 1]
                        nc.vector.tensor_scalar_mul(dst, src, r1)
                    tp = tpp.tile([128, 128], f32, tag="tp", name=f"tp{sb}_{pair}_{sc}")
                    nc.tensor.transpose(tp[:], cn[:], identf[:])
                    dstT = concatT[pair][:, s0b + sc * 128 : s0b + (sc + 1) * 128]
                    if sc % 2 == 0:
                        nc.scalar.copy(dstT, tp[:])
                    else:
                        nc.vector.tensor_copy(dstT, tp[:])

            emit_mask_dma(0, 0, gated=True)
            emit_vdma(0, gated=True)
            emit_vdma(1, gated=True)
            emit_mask_dma(0, 1, gated=True)
            for i in range(2):
                gate(wo[i][0:1, 0:2])
                nc.gpsimd.dma_start(wo[i][:], T["woT"][i * 128 : (i + 1) * 128, :])
            extras = [
                (lambda s=sbn, j=jt: emit_kproj_jt(s, j))
                for sbn in (1, 2, 3)
                for jt in range(2)
            ]
            po2L = None
            hist = []          # completed pairs: (sb, pair, Pt)
            for sb in range(NSB):
                for pair in range(2):
                    last_it = (sb == NSB - 1 and pair == 1)
                    if sb == 0:
                        extras.append(lambda t=2 * pair: emit_vproj_tb(t))
                        extras.append(lambda t=2 * pair + 1: emit_vproj_tb(t))

                    Pt = ptp.tile(
                        [128, 2 * NC_T * 512], bf, tag="Pt", name=f"Pt{sb}_{pair}"
                    )
                    pv = Pt[:].rearrange("p (c h s) -> p c h s", c=NC_T, h=2)
                    mv = mtiles[sb][:].rearrange("p (c s) -> p c s", c=NC_T)
                    pidx = sb * 2 + pair
                    if pidx >= 1:
                        dr = hist[-1]
                        # attnV out, s-partition orientation: 2 tiles x
                        # 2 s-chunks of [s, 130] at col offsets 0/256
                        po2 = [
                            bigp.tile([128, 512], f32, tag="big",
                                      name=f"av{dr[0]}_{dr[1]}_{i}")
                            for i in range(2)
                        ]

                    def emit_scores_h(ps, c, h2):
                        psl = slice(h2 * 64, h2 * 64 + 64)
                        nc.tensor.matmul(
                            ps[:, h2 * 512 : (h2 + 1) * 512],
                            kpT[pair][c // 8][psl, (c % 8) * 128 : (c % 8 + 1) * 128],
                            qpS[pair][sb][psl, :],
                            start=True, stop=True,
                        )

                    for c in range(NC_T):
                        ps = scp.tile(
                            [128, 1024], f32, tag="sc", name=f"sc{sb}_{pair}_{c}"
                        )
                        # burst-drain groups for this slot (emitted split
                        # around the dense scores matmuls below, which keeps
                        # the PE MAC duty high enough to hold HAM at 8/8)
                        gs = ()
                        psb = ppair = pPt = None
                        if pidx == 1 and c % 2 == 1 and c >= 9:
                            # first drained pair waits for vproj(2)/(3)
                            # (vpc 8-15 land at slots 5/7): bursts at 9-15
                            psb, ppair, pPt = hist[0][:3]
                            gs = (c - 9, c - 8)
                        elif pidx >= 2 and c % 2 == 1:
                            psb, ppair, pPt = hist[-1][:3]
                            gs = (c // 2,)

                        def burst(gidx, ks):
                            sc_, h2_ = gidx // 2, gidx % 2
                            h = ppair * 2 + h2_
                            tl = po2[sc_ // 2]
                            co = (sc_ % 2) * 256 + h2_ * 65
                            for k in ks:
                                pco = (2 * k + h2_) * 512 + sc_ * 128
                                nc.tensor.matmul(
                                    tl[:, co : co + 65],
                                    pPt[:, pco : pco + 128],
                                    vpc[k][:, h * 65 : h * 65 + 65],
                                    start=(k == 0), stop=(k == NC_T - 1),
                                )

                        emit_scores_h(ps, c, 0)
                        if gs:
                            burst(gs[0], range(0, 8) if len(gs) == 1
                                  else range(NC_T))
                        emit_scores_h(ps, c, 1)
                        if gs:
                            if len(gs) == 1:
                                burst(gs[0], range(8, NC_T))
                            else:
                                burst(gs[1], range(NC_T))

                        dve_c, pool_c = routes_for(sb)
                        plainL, plainG = _plain_n, PLAIN_GROUPS_N
                        if c in dve_c:
                            # fused exp2 bit-hack + additive mask on DVE:
                            # bits = int16(psum + maskB), reinterpreted bf16.
                            # psum is pre-scaled by ALPHA (folded into kproj);
                            # maskB = 16256 (unmasked) / 1024 (masked -> ~0).
                            for h2 in range(2):
                                nc.vector.tensor_tensor(
                                    Pt[:, (2 * c + h2) * 512 :
                                       (2 * c + h2 + 1) * 512].bitcast(i16),
                                    ps[:, h2 * 512 : (h2 + 1) * 512],
                                    mv[:, c, :], mybir.AluOpType.add,
                                )
                        else:
                            nc.scalar.activation(
                                Pt[:, c * 1024 : (c + 1) * 1024],
                                ps[:], Exp, scale=EXP_SCALE,
                                bias=biasqk[:, 4:5],
                            )
                        if c in pool_c:
                            # mask-mul on the otherwise-idle Pool engine
                            for h2 in range(2):
                                nc.gpsimd.tensor_tensor(
                                    pv[:, c, h2, :], pv[:, c, h2, :],
                                    mv[:, c, :], mybir.AluOpType.mult,
                                )
                        # DVE mask-mul timing for the plain ACT chunks
                        # (DVE_C chunks fold the mask; POOL_C handled above)
                        if last_it and c >= 12:
                            # last iteration: per-chunk mask so attnV can
                            # finish in-loop (no serial tail chain)
                            grp = (c, c + 1) if c in plainL else None
                        else:
                            grp = plainG.get(c)
                        if grp:
                            sl = slice(*grp)
                            for h2 in range(2):
                                nc.vector.tensor_mul(
                                    pv[:, sl, h2, :], pv[:, sl, h2, :],
                                    mv[:, sl, :],
                                )
                        if pidx >= 1 and c == NC_T - 1:
                            nsb_, npair_ = hist[-1][:2]
                            emit_norm(nsb_, npair_, po2)
                            if npair_ == 1:
                                for st in range(4):
                                    for mt in range(2):
                                        extras.append(
                                            lambda s=nsb_, a=st, b=mt:
                                            emit_wo_group(s, a, b)
                                        )
                        if c == 0 and pair == 1 and sb + 1 < NSB:
                            # mask prefetch here (not at pair 0) keeps the
                            # DMA-bound ramp free for the v blocks
                            emit_mask_dma(sb + 1, 0)
                            emit_mask_dma(sb + 1, 1)
                        if c == 1 and pair == 0 and sb + 1 < NSB:
                            emit_qdma(sb + 1)
                        elif c in (1, 3) and pair == 1 and sb + 1 < NSB:
                            emit_qproj_jt(sb + 1, c // 2)
                        elif extras and (
                            (pidx == 0 and c >= 2)
                            or (c % 2 == 1 and (
                                c >= 5 or (pair == 0 and c >= 3)
                                or sb == NSB - 1
                            ))
                        ):
                            extras.pop(0)()
                    hist.append((sb, pair, Pt))
            # tail: burst-drain the last pair, norm, remaining Wo groups
            psb, ppair, pPt = hist[-1]
            po2L = [
                bigp.tile([128, 512], f32, tag="big", name=f"avL_{i}")
                for i in range(2)
            ]
            for gidx in range(8):
                sc, h2 = gidx // 2, gidx % 2
                h = ppair * 2 + h2
                tl = po2L[sc // 2]
                co = (sc % 2) * 256 + h2 * 65
                for k in range(NC_T):
                    pco = (2 * k + h2) * 512 + sc * 128
                    nc.tensor.matmul(
                        tl[:, co : co + 65],
                        pPt[:, pco : pco + 128],
                        vpc[k][:, h * 65 : h * 65 + 65],
                        start=(k == 0), stop=(k == NC_T - 1),
                    )
                if extras:
                    # interleave pair-(3,0) wo groups: dense matmuls keep
                    # the PE duty (HAM) up between small-matmul bursts
                    extras.pop(0)()
            emit_norm(psb, ppair, po2L)
            for fn in extras:
                fn()
            for st in range(4):
                for mt in range(2):
                    emit_wo_group(NSB - 1, st, mt)


def build_nc():
    nc = bacc.Bacc("TRN2", target_bir_lowering=False, debug=False)
    names = {}
    def din(name, shape, dt):
        names[name] = nc.dram_tensor(name, shape, dt, kind="ExternalInput").ap()
    # All inputs pre-packed host-side into exact SBUF tile layout so every
    # DMA is a straight contiguous copy (8-16KB lines, minimal descriptors).
    din("qT", [NSB * 128, 8 * 512], f8)
    din("kT", [NSB * 128, 8 * 512], f8)
    din("vT", [NSB * 2 * 128, 8 * 256], f8)
    din("maskT", [NSB * 128, NC_T * 512], bf)
    din("wqT", [128, 8 * JC], f8)
    din("wkT", [128, 8 * JC], f8)
    din("wvT", [128, 8 * JC], f8)
    din("woT", [JC, D], bf)
    din("biasqk", [128, 5], f32)
    din("ident", [128, 128], f32)
    names["out_p"] = nc.dram_tensor(
        "out_p", [S, D], bf, kind="ExternalOutput"
    ).ap()
    with tile_mod.TileContext(nc) as tc:
        _emit(tc, names)
    nc.compile()
    return nc


_NC = None


def prep_inputs(q, k, v, mask, Wq, bq, Wk, bk, Wv, bv, Wo, bo):
    q = np.asarray(q, F32)
    k = np.asarray(k, F32)
    v = np.asarray(v, F32)
    mask = np.asarray(mask)
    Wq, Wk, Wv, Wo = (np.asarray(w, F32) for w in (Wq, Wk, Wv, Wo))
    bq, bk, bv, bo = (np.asarray(b_, F32) for b_ in (bq, bk, bv, bo))

    def pack_dS(xT, dt):
        # [D, S] -> [NSB*128, 8*512]: [sb*128+p, c*512+s] = xT[c*128+p, sb*512+s]
        x = xT.reshape(8, 128, NSB, 512)
        return np.ascontiguousarray(
            x.transpose(2, 1, 0, 3).reshape(NSB * 128, 8 * 512)
        ).astype(dt)

    def pack_dS_half(xT, dt):
        # t-half granularity: [tb2*128+p, c*256+s] = xT[c*128+p, tb2*256+s]
        x = xT.reshape(8, 128, NSB * 2, 256)
        return np.ascontiguousarray(
            x.transpose(2, 1, 0, 3).reshape(NSB * 2 * 128, 8 * 256)
        ).astype(dt)

    def pack_w(wT):
        # [D, JC] -> [128, 8*JC]: [p, c*JC+j] = wT[c*128+p, j]
        w = wT.reshape(8, 128, JC)
        return np.ascontiguousarray(w.transpose(1, 0, 2).reshape(128, 8 * JC))

    mT0 = mask[0, 0].T  # [t, s]
    m = (mT0 != 0).reshape(NC_T, 128, NSB, 512)
    # Route-dependent mask content per t-chunk c:
    #  c in DVE_C: additive bit-hack bias (16256 unmasked / 1024 masked -> P~0)
    #  else:       multiplicative 1/0
    is_dve = np.isin(np.arange(NC_T), list(DVE_C))[:, None, None, None]
    mvals = np.where(
        is_dve,
        np.where(m, np.float32(16256.0), np.float32(1024.0)),
        m.astype(np.float32),
    )
    maskT = np.ascontiguousarray(
        mvals.transpose(2, 1, 0, 3).reshape(NSB * 128, NC_T * 512)
    ).astype(BF16)
    qT = [pack_dS(q[b_].T, F8) for b_ in range(B)]
    kT = [pack_dS(k[b_].T, F8) for b_ in range(B)]
    vT = [pack_dS_half(v[b_].T, F8) for b_ in range(B)]

    in_maps = []
    for c in range(N_CORES):
        b_, g = c // 4, c % 4
        js = slice(g * JC, (g + 1) * JC)
        # k bias columns carry the ALPHA prefold (kproj scale = WINV*ALPHA);
        # col 4 = ACT_COMP exp bias (pwl route-bias compensation)
        biasqk = np.stack(
            [bq[js][:128], bq[js][128:],
             bk[js][:128] * ALPHA, bk[js][128:] * ALPHA,
             np.full(128, ACT_COMP, F32)], axis=1
        ).astype(F32)
        in_maps.append(
            {
                "qT": qT[b_],
                "kT": kT[b_],
                "vT": vT[b_],
                "maskT": maskT,
                "wqT": pack_w((Wq[js, :].T * WPRE)).astype(F8),
                "wkT": pack_w((Wk[js, :].T * WPRE)).astype(F8),
                "wvT": pack_w((Wv[js, :].T * WPRE)).astype(F8),
                "woT": np.ascontiguousarray(Wo[:, js].T).astype(BF16),
                "biasqk": np.ascontiguousarray(biasqk),
                "ident": np.eye(128, dtype=F32),
            }
        )
    # bv contributes a constant (softmax rows sum to 1): out += Wo @ bv + bo
    bias_out = (Wo @ bv + bo).astype(F32)
    return in_maps, bias_out


def run_prepped(in_maps, bias_out, trace=False, **kw):
    global _NC
    if _NC is None:
        _NC = build_nc()
    res = run_bass_kernel_spmd(
        _NC, in_maps, list(range(N_CORES)), trace=trace, **kw
    )
    out = np.zeros((B, S, D), F32)
    for c in range(N_CORES):
        out[c // 4] += np.asarray(res.results[c]["out_p"], dtype=F32)
    out += bias_out[None, None, :]
    return out, res


def kernel(q, k, v, mask, Wq, bq, Wk, bk, Wv, bv, Wo, bo):
    in_maps, bias_out = prep_inputs(
        q, k, v, mask, Wq, bq, Wk, bk, Wv, bv, Wo, bo
    )
    out, _ = run_prepped(in_maps, bias_out)
    return out

